# revision 1
# baseline (speedup 1.0000x reference)
# Trainium2 Bass kernel for nn_DiffNet.
#
# Math: the conv2(conv1(.)) meta-MLP is affine per element, so with
#   coef = (conv2_w @ conv1_w)[0]  (c0, c1, c2),
#   bc   = (conv2_w @ conv1_b)[0] + conv2_b[0],
#   scale = RATE / batch_num,
# each layer (W, b) of the reference reduces to
#   z  = vi @ W.T                      (pre-bias matmul)
#   vj = relu(z + b)
#   s  = rowsum(vi),  q = rowsum(vi^2)
#   out = (1 + C2*s) * vj + C1*z + (C0*q + Cb*s)
# with C* = scale * (c*, bc).  No [B, out, in] tensor is ever materialized.
#
# Sharding: data-parallel over batch (64 rows -> 8 rows/core), weights
# replicated per core, zero collectives.
#
# Device-side bias folding: PSUM holds P = vi' @ W.T + bhat, where inputs are
# represented as vi = vi' + m (m a constant row vector, m1 = 0) and
# bhat = b + m @ W.T, so P = z + b exactly.  Then
#   out' = alpha (.) relu(P) + C1*P + delta,   out = out' - C1*b,
# so the next layer's constant offset is m_next = -C1*b, folded on host into
# bhat_next, k_alpha, k_delta, and the q cross-term.
#
# Matmul operands are fp16 (4x PE rate vs fp32, half the HBM bytes);
# accumulation + epilogue stay fp32 (measured l2 rel err ~5e-4).
#
# Perf notes (from HW traces):
# - HWDGE descriptor-gen paces a queue at ~desc_size/20ns; per-partition
#   runs must be >=4KB, so all fp16 operands live in ONE [128, 7232] pack
#   (xt | w1 | w2 | w3) DMA'd in 4 column-slices on the sync queue while
#   pk1/pk8 ride the scalar queue.
# - PE HAM clock-gate: ~4us of warm-up matmuls on junk tiles first, so the
#   real matmuls run at 2.4GHz instead of 1.2.
# - Kernel tail pays ~115ns per semaphore reset: keep instruction count low
#   (fused delta reduction, single transpose-copy per boundary).

import numpy as np

RATE = 0.01
B, IN, H1, H2, OUT = 64, 1024, 512, 512, 256
NCORES = 8
BL = B // NCORES  # 8 rows per core
P128 = 128

# const columns in pk8: scalars, then per-layer [Cb, C0, 2C0] triples
C_C1, C_C2 = 0, 1
C_KA0 = 2    # 2,3,4 = k_alpha per layer
C_KD0 = 5    # 5,6,7 = k_delta per layer
C_ZERO = 8
C_TRI0 = 9   # 9..17: per-layer [Cb, C0, twoC0] (twoC0 = 0 for layer 0)
NCONST = 18

# pk1 (fp16, 1 partition): ones row | bhat1 | bhat2 | bhat3
PK1_ONES = 0
PK1_B = [8, 8 + H1, 8 + H1 + H2]
PK1_LEN = 8 + H1 + H2 + OUT

# pk8 (fp32, 8 partitions): x | m2r | m3r | m4r | cst | id8
PK8_X = 0
PK8_M = [None, IN, IN + H1]
PK8_M4 = IN + H1 + H2
PK8_CST = PK8_M4 + OUT
PK8_ID = PK8_CST + NCONST
PK8_LEN = PK8_ID + BL

# wall (fp16): xt | w1 chunks | w2 chunks | w3 chunks
XT_OFF = 0
XT_LEN = (IN // P128) * BL  # 64
W_OFF = [XT_LEN, XT_LEN + 4096, XT_LEN + 6144]
W_LEN = XT_LEN + 7168  # 7232

NKS = [IN // P128, H1 // P128, H2 // P128]
NOUTS = [H1, H2, OUT]

N_WARMUP = 10  # PE clock-gate warmup matmuls

_NC_CACHE = {}
DEBUG_TAPS = False


def _build_nc():
    import concourse.bacc as bacc
    import concourse.mybir as mybir
    import concourse.tile as tile

    fp32 = mybir.dt.float32
    fp16 = mybir.dt.float16
    AF = mybir.ActivationFunctionType
    ALU = mybir.AluOpType
    AX = mybir.AxisListType

    nc = bacc.Bacc("TRN2", target_bir_lowering=False, debug=False)

    pk1_t = nc.dram_tensor("pk1", [1, PK1_LEN], fp16, kind="ExternalInput")
    pk8_t = nc.dram_tensor("pk8", [BL, PK8_LEN], fp32, kind="ExternalInput")
    w_t = nc.dram_tensor("wall", [P128, W_LEN], fp16, kind="ExternalInput")
    out_t = nc.dram_tensor("outb", [BL, OUT], fp32, kind="ExternalOutput")

    with tile.TileContext(nc) as tc:
        with (
            tc.tile_pool(name="wp", bufs=1) as wp,
            tc.tile_pool(name="actp", bufs=1) as ap_,
            tc.tile_pool(name="scp", bufs=1) as scp,
            tc.tile_pool(name="pp", bufs=2, space="PSUM") as pp,
            tc.tile_pool(name="tpp", bufs=2, space="PSUM") as tpp,
        ):
            # --- PE warm-up: junk matmuls release the HAM clock gate ---
            junk_a = wp.tile([P128, BL], fp16, tag="junk_a")
            junk_w = wp.tile([P128, 512], fp16, tag="junk_w")
            nc.gpsimd.memset(junk_a[:], 0.0)
            nc.gpsimd.memset(junk_w[:], 0.0)
            warm_p = pp.tile([BL, 512], fp32, tag="warm")
            for _ in range(N_WARMUP):
                nc.tensor.matmul(
                    warm_p[:], junk_a[:, :BL], junk_w[:], start=True, stop=True
                )

            # --- DMAs (one serial completion chain per queue): wA leads so
            # L1 matmuls gate on the FIRST completion; pk8 second (stats are
            # needed mid-L1); pk1 after wB (bias row is the last L1 matmul).
            wseg = []  # (tile, col offset within wall)

            def wdma(name, lo, hi):
                t = wp.tile([P128, hi - lo], fp16, tag=name)
                nc.sync.dma_start(t[:], w_t[:, lo:hi])
                wseg.append((t, lo))

            wdma("wA", 0, 2624)           # xt + w1 chunks 0-4
            pk8 = ap_.tile([BL, PK8_LEN], fp32, tag="pk8")
            nc.sync.dma_start(pk8[:], pk8_t[:])
            wdma("wB", 2624, 4160)        # w1 chunks 5-7
            pk1 = ap_.tile([1, PK1_LEN], fp16, tag="pk1")
            nc.sync.dma_start(pk1[:], pk1_t[:])
            wdma("wC", 4160, 6208)        # w2
            wdma("wD", 6208, 7232)        # w3

            def wall_slice(lo, n):
                for t, off in wseg:
                    if off <= lo and lo + n <= off + t.shape[1]:
                        return t[:, lo - off : lo - off + n]
                raise AssertionError("bad wall slice")

            x_s = pk8[:, PK8_X : PK8_X + IN]
            id_s = pk8[:, PK8_ID : PK8_ID + BL]

            def col(j):
                c = PK8_CST + j
                return pk8[:, c : c + 1]

            # lhsT chunk slices per layer (fp16 [128, BL] each)
            vt = [[wall_slice(XT_OFF + k * BL, BL) for k in range(NKS[0])]]

            def layer(l, svec):
                """svec: {"sv": [BL,3] s|q tile, "ce": fused-delta operand}."""
                nk, nout = NKS[l], NOUTS[l]
                # alpha = C2*s + k_alpha
                al = scp.tile([BL, 1], fp32, tag=f"al{l}")
                nc.vector.tensor_scalar(
                    al[:], svec["sv"][:, 0:1], col(C_C2), col(C_KA0 + l),
                    ALU.mult, ALU.add
                )
                # delta = sum over crs_ext + k_delta.  For l>0, crs_ext is
                # [o*(2C0*m) | Cb*s | C0*q] (the cross block written by gpsimd
                # at the boundary); for l=0 only the [Cb*s | C0*q] tail exists.
                ce = svec["ce"]
                nc.vector.tensor_tensor(
                    ce[:, -2:],
                    svec["sv"][:, 0:2],
                    pk8[:, PK8_CST + C_TRI0 + 3 * l : PK8_CST + C_TRI0 + 3 * l + 2],
                    ALU.mult,
                )
                de = scp.tile([BL, 1], fp32, tag=f"de{l}")
                nc.vector.tensor_reduce(
                    out=de[:], in_=ce[:], axis=AX.X, op=ALU.add
                )
                de2 = scp.tile([BL, 1], fp32, tag=f"de2{l}")
                nc.vector.tensor_scalar(
                    de2[:], de[:], col(C_KD0 + l), None, ALU.add
                )
                # P = vi' @ W.T + bhat
                Pt = pp.tile([BL, nout], fp32, tag="P")
                for k in range(nk):
                    nc.tensor.matmul(
                        Pt[:],
                        vt[l][k],
                        wall_slice(W_OFF[l] + k * nout, nout),
                        start=(k == 0),
                        stop=False,
                    )
                boff = PK1_B[l]
                nc.tensor.matmul(
                    Pt[:],
                    pk1[:, PK1_ONES : PK1_ONES + BL],
                    pk1[:, boff : boff + nout],
                    start=False,
                    stop=True,
                )

                # epilogue: out' = relu(P*alpha) + (C1*P + delta)   [alpha > 0]
                vja = ap_.tile([BL, nout], fp32, tag=f"vja{l}")
                relu_inst = nc.scalar.activation(
                    out=vja[:], in_=Pt[:], func=AF.Relu, scale=al[:, 0:1],
                    bias=col(C_ZERO),
                )
                if l < 2:
                    # pinned keep-warm: ordered after the relu (sync=False =>
                    # no runtime wait) so the scheduler can't hoist them; they
                    # dispatch right after this layer's matmuls and keep the
                    # PE HAM clock-gate open through the epilogue gap
                    from concourse.tile_rust import add_dep_helper
                    for _ in range(8):
                        ji = nc.tensor.matmul(
                            warm_p[:], junk_a[:, :BL], junk_w[:],
                            start=True, stop=True,
                        )
                        add_dep_helper(
                            ji.ins, relu_inst.ins, sync=False,
                            reason="pin keep-warm after relu",
                        )
                tC = ap_.tile([BL, nout], fp32, tag=f"tC{l}")
                nc.vector.tensor_scalar(
                    tC[:], Pt[:], col(C_C1), de2[:, 0:1], ALU.mult, ALU.add
                )
                if l == 2:
                    # out = (vja + m4) + tC; the m4 add runs on gpsimd in
                    # parallel with tC on vector
                    gv = ap_.tile([BL, nout], fp32, tag="gv")
                    nc.gpsimd.tensor_tensor(
                        gv[:], vja[:], pk8[:, PK8_M4 : PK8_M4 + OUT], ALU.add
                    )
                    o = ap_.tile([BL, nout], fp32, tag=f"o{l}")
                    nc.vector.tensor_tensor(o[:], gv[:], tC[:], ALU.add)
                    return o, None
                o = ap_.tile([BL, nout], fp32, tag=f"o{l}")
                nc.vector.tensor_tensor(o[:], vja[:], tC[:], ALU.add)
                # transposes -> next layer's fp16 lhsT chunks (one copy)
                nch = nout // P128
                tp = tpp.tile([P128, nch * BL], fp32, tag="tp")
                for c in range(nch):
                    nc.tensor.transpose(
                        tp[:, c * BL : (c + 1) * BL],
                        o[:, c * P128 : (c + 1) * P128],
                        id_s,
                    )
                vtn = ap_.tile([P128, nch * BL], fp16, tag=f"vt{l + 1}")
                # explicit DVE: on ACT this copy queues behind the Square stat
                # (readiness order), delaying the next layer's matmuls.
                # Two half-copies so the next layer's first matmuls start
                # while the second half is still copying.
                h = (nch // 2) * BL
                nc.vector.tensor_copy(out=vtn[:, :h], in_=tp[:, :h])
                nc.vector.tensor_copy(out=vtn[:, h:], in_=tp[:, h:])
                vt.append([vtn[:, k * BL : (k + 1) * BL] for k in range(nch)])
                # next-layer stats; the cross products go on the idle
                # gpsimd engine straight into the fused-delta operand, and
                # Square (ACT) goes last so it cannot delay the transpose copy
                sv = scp.tile([BL, 3], fp32, tag=f"sv{l + 1}")
                nc.vector.reduce_sum(out=sv[:, 0:1], in_=o[:], axis=AX.X)
                ce = scp.tile([BL, nout + 2], fp32, tag=f"ce{l + 1}")
                nc.gpsimd.tensor_tensor(
                    ce[:, :nout], o[:],
                    pk8[:, PK8_M[l + 1] : PK8_M[l + 1] + nout], ALU.mult
                )
                sq = scp.tile([BL, nout], fp32, tag=f"sq{l + 1}")
                nc.scalar.activation(
                    out=sq[:], in_=o[:], func=AF.Square, bias=col(C_ZERO),
                    accum_out=sv[:, 1:2],
                )
                return o, {"sv": sv, "ce": ce[:, : nout + 2]}

            # layer-1 stats straight from fp32 x
            sv1 = scp.tile([BL, 3], fp32, tag="sv1")
            nc.vector.reduce_sum(out=sv1[:, 0:1], in_=x_s, axis=AX.X)
            sq0 = scp.tile([BL, IN], fp32, tag="sq0")
            nc.scalar.activation(
                out=sq0[:], in_=x_s, func=AF.Square, bias=col(C_ZERO),
                accum_out=sv1[:, 1:2],
            )
            ce1 = scp.tile([BL, 2], fp32, tag="ce1")

            o1, sv2 = layer(0, {"sv": sv1, "ce": ce1})
            o2, sv3 = layer(1, sv2)
            o3, _ = layer(2, sv3)

            nc.sync.dma_start(out_t[:], o3[:])

            if DEBUG_TAPS:
                for name, ap in (("dbg_o1", o1[:]), ("dbg_o2", o2[:])):
                    t = nc.dram_tensor(
                        name, list(ap.shape), ap.dtype, kind="ExternalOutput"
                    )
                    nc.sync.dma_start(t[:], ap)

    nc.compile()
    return nc


def get_nc():
    if "nc" not in _NC_CACHE:
        _NC_CACHE["nc"] = _build_nc()
    return _NC_CACHE["nc"]


def _chunk_pt(a, dtype):
    """[R, C] -> [128, (R//128)*C]: row-chunks of 128 side by side."""
    r, c = a.shape
    nk = r // P128
    return np.ascontiguousarray(
        a.reshape(nk, P128, c).transpose(1, 0, 2).reshape(P128, nk * c), dtype=dtype
    )


def host_prep(x, fc1_w, fc1_b, fc2_w, fc2_b, fc3_w, fc3_b,
              conv1_w, conv1_b, conv2_w, conv2_b, batch_num):
    f32, f16 = np.float32, np.float16
    x = np.asarray(x, f32)
    fc1_w = np.asarray(fc1_w, f32)
    fc2_w = np.asarray(fc2_w, f32)
    fc3_w = np.asarray(fc3_w, f32)
    fc1_b = np.asarray(fc1_b, f32)
    fc2_b = np.asarray(fc2_b, f32)
    fc3_b = np.asarray(fc3_b, f32)

    bn = float(np.asarray(batch_num).item())
    scale = RATE / bn
    coef = (np.asarray(conv2_w, np.float64) @ np.asarray(conv1_w, np.float64))[0]
    bc = float(
        (np.asarray(conv2_w, np.float64) @ np.asarray(conv1_b, np.float64))[0]
        + np.asarray(conv2_b, np.float64)[0]
    )
    C0, C1, C2 = (scale * coef).astype(np.float64)
    Cb = scale * bc

    m2 = (-C1 * fc1_b.astype(np.float64)).astype(f32)
    m3 = (-C1 * fc2_b.astype(np.float64)).astype(f32)
    m4 = (-C1 * fc3_b.astype(np.float64)).astype(f32)
    bh1 = fc1_b
    bh2 = (fc2_b + m2 @ fc2_w.T).astype(f32)
    bh3 = (fc3_b + m3 @ fc3_w.T).astype(f32)

    ka = [1.0, 1.0 + C2 * float(m2.sum()), 1.0 + C2 * float(m3.sum())]
    kd = [
        0.0,
        C0 * float(m2 @ m2) + Cb * float(m2.sum()),
        C0 * float(m3 @ m3) + Cb * float(m3.sum()),
    ]
    cvec = np.zeros(NCONST, dtype=f32)
    cvec[C_C1], cvec[C_C2] = C1, C2
    cvec[C_KA0 : C_KA0 + 3] = ka
    cvec[C_KD0 : C_KD0 + 3] = kd
    for l in range(3):
        cvec[C_TRI0 + 3 * l : C_TRI0 + 3 * l + 3] = [
            Cb, C0, 0.0 if l == 0 else 2 * C0
        ]

    pk1 = np.zeros((1, PK1_LEN), f16)
    pk1[0, PK1_ONES : PK1_ONES + BL] = 1.0
    pk1[0, PK1_B[0] : PK1_B[0] + H1] = bh1.astype(f16)
    pk1[0, PK1_B[1] : PK1_B[1] + H2] = bh2.astype(f16)
    pk1[0, PK1_B[2] : PK1_B[2] + OUT] = bh3.astype(f16)

    wall_base = np.empty((P128, W_LEN), f16)
    wall_base[:, W_OFF[0] : W_OFF[0] + 4096] = _chunk_pt(fc1_w.T, f16)
    wall_base[:, W_OFF[1] : W_OFF[1] + 2048] = _chunk_pt(fc2_w.T, f16)
    wall_base[:, W_OFF[2] : W_OFF[2] + 1024] = _chunk_pt(fc3_w.T, f16)

    pk8_base = np.zeros((BL, PK8_LEN), f32)
    pk8_base[:, PK8_M[1] : PK8_M[1] + H1] = (2.0 * C0 * m2.astype(np.float64)).astype(f32)
    pk8_base[:, PK8_M[2] : PK8_M[2] + H2] = (2.0 * C0 * m3.astype(np.float64)).astype(f32)
    pk8_base[:, PK8_M4 : PK8_M4 + OUT] = m4
    pk8_base[:, PK8_CST : PK8_CST + NCONST] = cvec
    pk8_base[:, PK8_ID : PK8_ID + BL] = np.eye(BL, dtype=f32)

    in_maps = []
    for k in range(NCORES):
        xk = np.ascontiguousarray(x[k * BL : (k + 1) * BL], dtype=f32)
        pk8 = pk8_base.copy()
        pk8[:, PK8_X : PK8_X + IN] = xk
        wall = wall_base.copy()
        wall[:, XT_OFF : XT_OFF + XT_LEN] = _chunk_pt(xk.T.copy(), f16)
        in_maps.append({"pk1": pk1, "pk8": pk8, "wall": wall})
    return in_maps


def kernel(**inputs):
    from concourse.bass_utils import run_bass_kernel_spmd

    nc = get_nc()
    in_maps = host_prep(**inputs)
    res = run_bass_kernel_spmd(nc, in_maps, core_ids=list(range(NCORES)))
    out = np.concatenate([res.results[k]["outb"] for k in range(NCORES)], axis=0)
    return np.ascontiguousarray(out, dtype=np.float32)



# revision 10
# speedup vs baseline: 1.0782x; 1.0782x over previous
# Trainium2 Bass kernel for nn_DiffNet.
#
# Math: the conv2(conv1(.)) meta-MLP is affine per element, so with
#   coef = (conv2_w @ conv1_w)[0]  (c0, c1, c2),
#   bc   = (conv2_w @ conv1_b)[0] + conv2_b[0],
#   scale = RATE / batch_num,
# each layer (W, b) of the reference reduces to
#   z  = vi @ W.T                      (pre-bias matmul)
#   vj = relu(z + b)
#   s  = rowsum(vi),  q = rowsum(vi^2)
#   out = (1 + C2*s) * vj + C1*z + (C0*q + Cb*s)
# with C* = scale * (c*, bc).  No [B, out, in] tensor is ever materialized.
#
# Sharding: data-parallel over batch (64 rows -> 8 rows/core), weights
# replicated per core, zero collectives.
#
# Key fusions vs the v0 kernel:
# - alpha*relu(P) + C1*P == LeakyRelu_k(alpha' * P) with alpha' = alpha + C1
#   and per-row slope k = C1/alpha' (alpha' ~ 1 > 0) -> the whole per-layer
#   epilogue is ONE scalar-engine activation; its accum_out gives rowsum(o)
#   for free.
# - the per-row constant de2 (delta) is folded into the NEXT layer as a
#   rank-1 update: P_next += de2 (x) colsum(W_next), via a K=2 bias matmul
#   [ones; de2] @ [bhat; wsum].
# - activations are fp16 out of the ACT, transposed on the PE in fp16
#   (PSUM fp16), single copy to SBUF as the next layer's lhsT.
# - layer-1 alpha/k/delta depend only on x -> computed on host, shipped in
#   pk8.
# - w1 is DMA'd in 3 column slices so L1 matmul chunk k starts as soon as
#   its slice lands (pipelines the dominant weight DMA with the PE).
#
# Device-side bias folding: P = vi' @ W.T + bhat, where inputs are
# represented as vi = vi' + m (m = -C1*b_prev, m1 = 0) and
# bhat = b + m @ W.T, so P = z + b exactly.
#
# Perf notes (from HW traces):
# - HWDGE descriptor-gen paces a queue at ~650ns/instr; 16 DMA engines
#   stream ~265GB/s aggregate; first packet lands ~1.5us after the trigger.
# - PE HAM clock-gate: warm-up matmuls on junk tiles first, plus pinned
#   keep-warm matmuls after each boundary ACT.
# - Kernel tail pays a fixed ~250-sem reset sequence (~6us) regardless.

import numpy as np

RATE = 0.01
B, IN, H1, H2, OUT = 64, 1024, 512, 512, 256
NCORES = 8
BL = B // NCORES  # 8 rows per core
P128 = 128

# ---- pk8 (fp32, 8 partitions) column map ----
P8_ALPHA1 = 0   # per-row alpha'_1
P8_K1 = 1       # per-row k_1
P8_DE21 = 2     # per-row de2_1
P8_C1 = 3
P8_C0 = 4
P8_CB = 5
P8_C0C1X2 = 6   # 2*C0*C1
P8_C0X2 = 7     # 2*C0
P8_C0N = 8      # C0*512
P8_C2 = 9
P8_C2N = 10     # C2*512
P8_KD = {2: 11, 3: 12}        # kd'_l = C0*sum(m^2)+Cb*sum(m)
P8_CBNCM = {2: 13, 3: 14}     # Cb*n + 2*C0*sum(m)
P8_KA = {2: 15, 3: 16}        # 1 + C2*sum(m) + C1
P8_M4REP = 24   # [8, 256] m4 replicated
PK8_LEN = P8_M4REP + OUT

# ---- pk2 (fp16, 8 partitions) column map ----
K2_LHS2 = 0     # [2,8]: p0 ones, p1 de2_1 (host)
K2_LHS3 = 8     # [2,8]: p0 ones, p1 de2_2 (device-written slot)
K2_RHS2 = 16    # [2,512]: p0 bhat2, p1 wsum2
K2_RHS3 = K2_RHS2 + H2            # [2,256]: p0 bhat3, p1 wsum3
K2_BH1 = K2_RHS3 + OUT            # [1,512] bhat1
K2_ID8 = K2_BH1 + H1              # [8,8] identity
K2_NB1 = K2_ID8 + 8               # [8,512] -b1 replicated (o1 cross term)
K2_NB2 = K2_NB1 + H1              # [8,512] -b2 replicated (o2 cross term)
PK2_LEN = K2_NB2 + H2

# ---- wall (fp16): xt | w1 chunks | w2 chunks | w3 chunks ----
XT_OFF = 0
XT_LEN = (IN // P128) * BL  # 64
W_OFF = [XT_LEN, XT_LEN + 4096, XT_LEN + 6144]
W_LEN = XT_LEN + 7168  # 7232
# DMA split points (sync queue, in priority order)
W_SPLITS = [(0, 1088), (1088, 2624), (2624, 4160), (4160, 6208), (6208, 7232)]

NKS = [IN // P128, H1 // P128, H2 // P128]
NOUTS = [H1, H2, OUT]

N_WARMUP = 4       # PE clock-gate warmup matmuls (N=512)
N_KEEPWARM = 5     # pinned junk matmuls (N=128) after each boundary ACT

_NC_CACHE = {}
DEBUG_TAPS = False


def _build_nc():
    import concourse.bacc as bacc
    import concourse.mybir as mybir
    import concourse.tile as tile
    from concourse.tile_rust import add_dep_helper

    fp32 = mybir.dt.float32
    fp16 = mybir.dt.float16
    AF = mybir.ActivationFunctionType
    ALU = mybir.AluOpType
    AX = mybir.AxisListType

    nc = bacc.Bacc("TRN2", target_bir_lowering=False, debug=False)

    pk8_t = nc.dram_tensor("pk8", [BL, PK8_LEN], fp32, kind="ExternalInput")
    pk2_t = nc.dram_tensor("pk2", [BL, PK2_LEN], fp16, kind="ExternalInput")
    w_t = nc.dram_tensor("wall", [P128, W_LEN], fp16, kind="ExternalInput")
    out_t = nc.dram_tensor("outb", [BL, OUT], fp32, kind="ExternalOutput")

    with tile.TileContext(nc) as tc:
        with (
            tc.tile_pool(name="wp", bufs=1) as wp,
            tc.tile_pool(name="actp", bufs=1) as ap_,
            tc.tile_pool(name="scp", bufs=1) as scp,
            tc.tile_pool(name="pp", bufs=2, space="PSUM") as pp,
            tc.tile_pool(name="tpp", bufs=2, space="PSUM") as tpp,
        ):
            # --- PE warm-up: junk matmuls release the HAM clock gate ---
            junk_a = wp.tile([P128, BL], fp16, tag="junk_a")
            junk_w = wp.tile([P128, 512], fp16, tag="junk_w")
            nc.gpsimd.memset(junk_a[:], 0.0)
            nc.gpsimd.memset(junk_w[:], 0.0)
            warm_p = pp.tile([BL, 512], fp32, tag="warm")
            for _ in range(N_WARMUP):
                nc.tensor.matmul(
                    warm_p[:], junk_a[:, :BL], junk_w[:], start=True, stop=True
                )

            # --- DMAs: wall slices on sync (priority order), pk8/pk2 on
            # the scalar HWDGE queue so they don't delay w1.
            wseg = []
            for i, (lo, hi) in enumerate(W_SPLITS):
                t = wp.tile([P128, hi - lo], fp16, tag=f"w{i}")
                nc.sync.dma_start(t[:], w_t[:, lo:hi])
                wseg.append((t, lo))
            pk8 = ap_.tile([BL, PK8_LEN], fp32, tag="pk8")
            nc.scalar.dma_start(pk8[:], pk8_t[:])
            pk2 = ap_.tile([BL, PK2_LEN], fp16, tag="pk2")
            nc.scalar.dma_start(pk2[:], pk2_t[:])

            def wall_slice(lo, n):
                for t, off in wseg:
                    if off <= lo and lo + n <= off + t.shape[1]:
                        return t[:, lo - off : lo - off + n]
                raise AssertionError("bad wall slice")

            def col8(j):
                return pk8[:, j : j + 1]

            id8 = pk2[:, K2_ID8 : K2_ID8 + 8]

            # L1 lhsT chunks (fp16 [128, BL] each) from the wall
            vt = [[wall_slice(XT_OFF + k * BL, BL) for k in range(NKS[0])]]

            sqjunk = scp.tile([BL, H1], fp16, tag="sqjunk")
            cejunk = scp.tile([BL, H1], fp16, tag="cejunk")

            def layer_mms(l, lhs_bias, rhs_bias):
                """Accumulate P_l = vi' @ W.T (+ bias rank-1/2) in PSUM."""
                nk, nout = NKS[l], NOUTS[l]
                Pt = pp.tile([BL, nout], fp32, tag="P")
                for k in range(nk):
                    nc.tensor.matmul(
                        Pt[:],
                        vt[l][k],
                        wall_slice(W_OFF[l] + k * nout, nout),
                        start=(k == 0),
                        stop=False,
                    )
                nc.tensor.matmul(Pt[:], lhs_bias, rhs_bias, start=False, stop=True)
                return Pt

            def boundary(l, Pt, alpha_ap, k_ap, nb_off):
                """LeakyRelu epilogue + fp16 transpose to next lhsT + stats.

                Returns (vtn chunks, s_core, q_core, red_ce, oc).
                """
                nout = NOUTS[l]
                nch = nout // P128
                oc = ap_.tile([BL, nout], fp16, tag=f"oc{l}")
                s_core = scp.tile([BL, 2], fp32, tag=f"sq{l}")
                act = nc.scalar.activation(
                    out=oc[:], in_=Pt[:], func=AF.Lrelu,
                    scale=alpha_ap, alpha=k_ap, bias=0.0,
                    accum_out=s_core[:, 0:1],
                )
                # pinned keep-warm: keep the PE HAM clock-gate open through
                # the epilogue gap without delaying next-layer matmuls
                for _ in range(N_KEEPWARM):
                    ji = nc.tensor.matmul(
                        warm_p[:, :P128], junk_a[:, :BL], junk_w[:, :P128],
                        start=True, stop=True,
                    )
                    add_dep_helper(
                        ji.ins, act.ins, sync=False,
                        reason="pin keep-warm after act",
                    )
                # fp16 PE transposes straight into PSUM, then one copy out
                tp = tpp.tile([P128, nch * BL], fp16, tag="tp")
                for c in range(nch):
                    nc.tensor.transpose(
                        tp[:, c * BL : (c + 1) * BL],
                        oc[:, c * P128 : (c + 1) * P128],
                        id8,
                    )
                vtn = ap_.tile([P128, nch * BL], fp16, tag=f"vt{l + 1}")
                h = (nch // 2) * BL if nch > 1 else nch * BL
                nc.vector.tensor_copy(out=vtn[:, :h], in_=tp[:, :h])
                if h < nch * BL:
                    nc.vector.tensor_copy(out=vtn[:, h:], in_=tp[:, h:])
                vt.append([vtn[:, k * BL : (k + 1) * BL] for k in range(nch)])
                # stats: q = rowsum(oc^2) via ACT accum; cross = rowsum(oc*(-b))
                # via gpsimd scalar_tensor_tensor accum
                nc.scalar.activation(
                    out=sqjunk[:, :nout], in_=oc[:], func=AF.Square, bias=0.0,
                    accum_out=s_core[:, 1:2],
                )
                red_ce = scp.tile([BL, 1], fp32, tag=f"ce{l}")
                nc.vector.scalar_tensor_tensor(
                    out=cejunk[:, :nout], in0=oc[:], scalar=1.0,
                    in1=pk2[:, nb_off : nb_off + nout],
                    op0=ALU.mult, op1=ALU.mult, accum_out=red_ce[:],
                )
                return s_core, red_ce, oc

            def fixups(l_next, s_core, red_ce, de2_prev):
                """alpha'/k/de2 for layer l_next from boundary stats (DVE).

                alpha/k chain first (needs only s, ready right after the
                Lrelu accum read) so the next ACT is never gated on the
                q statistic.
                """
                t = scp.tile([BL, 6], fp32, tag=f"fx{l_next}")
                s_ = s_core[:, 0:1]
                q_ = s_core[:, 1:2]
                t1, t2, base, u1, u, v = (t[:, i : i + 1] for i in range(6))
                a = scp.tile([BL, 2], fp32, tag=f"ak{l_next}")
                nc.vector.tensor_scalar(
                    a[:, 0:1], s_, col8(P8_C2), col8(P8_KA[l_next]),
                    ALU.mult, ALU.add)
                alphan = scp.tile([BL, 1], fp32, tag=f"al{l_next}")
                nc.vector.tensor_scalar(
                    alphan[:], de2_prev, col8(P8_C2N), a[:, 0:1],
                    ALU.mult, ALU.add)
                kn = scp.tile([BL, 1], fp32, tag=f"k{l_next}")
                nc.vector.reciprocal(a[:, 1:2], alphan[:])
                nc.vector.tensor_scalar(
                    kn[:], a[:, 1:2], col8(P8_C1), None, ALU.mult)
                nc.vector.tensor_scalar(
                    t1, q_, col8(P8_C0), col8(P8_KD[l_next]), ALU.mult, ALU.add)
                nc.vector.tensor_scalar(
                    t2, s_, col8(P8_CB), t1, ALU.mult, ALU.add)
                nc.vector.tensor_scalar(
                    base, red_ce[:], col8(P8_C0C1X2), t2, ALU.mult, ALU.add)
                nc.vector.tensor_scalar(
                    u1, s_, col8(P8_C0X2), col8(P8_CBNCM[l_next]),
                    ALU.mult, ALU.add)
                nc.vector.tensor_scalar(
                    u, de2_prev, col8(P8_C0N), u1, ALU.mult, ALU.add)
                nc.vector.tensor_tensor(v, de2_prev, u, ALU.mult)
                de2n = scp.tile([BL, 1], fp32, tag=f"de2_{l_next}")
                nc.vector.tensor_tensor(de2n[:], base, v, ALU.add)
                return alphan, kn, de2n

            # ---------- layer 1 ----------
            P1 = layer_mms(
                0, pk2[0:1, K2_LHS2 : K2_LHS2 + BL],
                pk2[0:1, K2_BH1 : K2_BH1 + H1],
            )
            s1, ce1, oc1 = boundary(0, P1, col8(P8_ALPHA1), col8(P8_K1), K2_NB1)
            al2, k2, de2_2 = fixups(2, s1, ce1, col8(P8_DE21))

            # ---------- layer 2 ----------
            P2 = layer_mms(
                1, pk2[0:2, K2_LHS2 : K2_LHS2 + BL],
                pk2[0:2, K2_RHS2 : K2_RHS2 + H2],
            )

            # [ones; de2_2] -> fp16 [2,8] lhsT for L3's K=2 bias mm, built
            # via one PE transpose of [8,2] (engines can't address a
            # partition-1 base directly).  Emitted AFTER the L2 matmuls:
            # the PE queue is in-order, so an earlier transpose would
            # stall L2 on the fixups.
            de2pair = scp.tile([BL, 2], fp16, tag="de2pair")
            nc.vector.memset(de2pair[:, 0:1], 1.0)
            nc.vector.tensor_copy(out=de2pair[:, 1:2], in_=de2_2[:])
            de2T = tpp.tile([2, BL], fp16, tag="de2T")
            nc.tensor.transpose(de2T[:], de2pair[:], id8)
            lhs3 = scp.tile([2, BL], fp16, tag="lhs3")
            nc.vector.tensor_copy(out=lhs3[:], in_=de2T[:])

            s2, ce2, oc2 = boundary(1, P2, al2[:], k2[:], K2_NB2)
            al3, k3, de2_3 = fixups(3, s2, ce2, de2_2[:])

            # m4d = m4rep + de2_3 (off critical path, during L3 matmuls)
            m4d = ap_.tile([BL, OUT], fp32, tag="m4d")
            nc.vector.tensor_scalar(
                m4d[:], pk8[:, P8_M4REP : P8_M4REP + OUT], de2_3[:], None,
                ALU.add)

            # ---------- layer 3 ----------
            P3 = layer_mms(
                2, lhs3[:],
                pk2[0:2, K2_RHS3 : K2_RHS3 + OUT],
            )
            oc3 = ap_.tile([BL, OUT], fp32, tag="oc3")
            act3 = nc.scalar.activation(
                out=oc3[:], in_=P3[:], func=AF.Lrelu,
                scale=al3[:], alpha=k3[:], bias=0.0,
            )
            o3 = ap_.tile([BL, OUT], fp32, tag="o3")
            nc.vector.tensor_tensor(o3[:], oc3[:], m4d[:], ALU.add)

            nc.sync.dma_start(out_t[:], o3[:])

            if DEBUG_TAPS:
                for name, ap in (("dbg_o1", oc1[:]), ("dbg_o2", oc2[:])):
                    t = nc.dram_tensor(
                        name, list(ap.shape), ap.dtype, kind="ExternalOutput"
                    )
                    nc.sync.dma_start(t[:], ap)

    nc.compile()
    return nc


def get_nc():
    if "nc" not in _NC_CACHE:
        _NC_CACHE["nc"] = _build_nc()
    return _NC_CACHE["nc"]


def _chunk_pt(a, dtype):
    """[R, C] -> [128, (R//128)*C]: row-chunks of 128 side by side."""
    r, c = a.shape
    nk = r // P128
    return np.ascontiguousarray(
        a.reshape(nk, P128, c).transpose(1, 0, 2).reshape(P128, nk * c), dtype=dtype
    )


def host_prep(x, fc1_w, fc1_b, fc2_w, fc2_b, fc3_w, fc3_b,
              conv1_w, conv1_b, conv2_w, conv2_b, batch_num):
    f32, f16, f64 = np.float32, np.float16, np.float64
    x = np.asarray(x, f32)
    fc1_w = np.asarray(fc1_w, f32)
    fc2_w = np.asarray(fc2_w, f32)
    fc3_w = np.asarray(fc3_w, f32)
    b1 = np.asarray(fc1_b, f64)
    b2 = np.asarray(fc2_b, f64)
    b3 = np.asarray(fc3_b, f64)

    bn = float(np.asarray(batch_num).item())
    scale = RATE / bn
    coef = (np.asarray(conv2_w, f64) @ np.asarray(conv1_w, f64))[0]
    bc = float(
        (np.asarray(conv2_w, f64) @ np.asarray(conv1_b, f64))[0]
        + np.asarray(conv2_b, f64)[0]
    )
    C0, C1, C2 = (scale * coef).astype(f64)
    Cb = scale * bc

    m2 = -C1 * b1
    m3 = -C1 * b2
    m4 = (-C1 * b3).astype(f32)
    bh1 = b1
    bh2 = b2 + m2 @ fc2_w.T.astype(f64)
    bh3 = b3 + m3 @ fc3_w.T.astype(f64)
    wsum2 = fc2_w.astype(f64).sum(axis=1)
    wsum3 = fc3_w.astype(f64).sum(axis=1)

    # ---- pk8 base (constants identical across cores) ----
    pk8_base = np.zeros((BL, PK8_LEN), f32)

    def setc(j, v):
        pk8_base[:, j] = v

    setc(P8_C1, C1)
    setc(P8_C0, C0)
    setc(P8_CB, Cb)
    setc(P8_C0C1X2, 2.0 * C0 * C1)
    setc(P8_C0X2, 2.0 * C0)
    setc(P8_C0N, C0 * 512.0)
    setc(P8_C2, C2)
    setc(P8_C2N, C2 * 512.0)
    for l, m in ((2, m2), (3, m3)):
        setc(P8_KD[l], C0 * float(m @ m) + Cb * float(m.sum()))
        setc(P8_CBNCM[l], Cb * 512.0 + 2.0 * C0 * float(m.sum()))
        setc(P8_KA[l], 1.0 + C2 * float(m.sum()) + C1)
    pk8_base[:, P8_M4REP : P8_M4REP + OUT] = m4

    # ---- pk2 base ----
    pk2_base = np.zeros((BL, PK2_LEN), f16)
    pk2_base[0, K2_LHS2 : K2_LHS2 + BL] = 1.0
    pk2_base[0, K2_LHS3 : K2_LHS3 + BL] = 1.0
    pk2_base[0, K2_RHS2 : K2_RHS2 + H2] = bh2.astype(f16)
    pk2_base[1, K2_RHS2 : K2_RHS2 + H2] = wsum2.astype(f16)
    pk2_base[0, K2_RHS3 : K2_RHS3 + OUT] = bh3.astype(f16)
    pk2_base[1, K2_RHS3 : K2_RHS3 + OUT] = wsum3.astype(f16)
    pk2_base[0, K2_BH1 : K2_BH1 + H1] = bh1.astype(f16)
    pk2_base[:, K2_ID8 : K2_ID8 + 8] = np.eye(BL, dtype=f16)
    pk2_base[:, K2_NB1 : K2_NB1 + H1] = (-b1).astype(f16)
    pk2_base[:, K2_NB2 : K2_NB2 + H2] = (-b2).astype(f16)

    wall_base = np.empty((P128, W_LEN), f16)
    wall_base[:, W_OFF[0] : W_OFF[0] + 4096] = _chunk_pt(fc1_w.T, f16)
    wall_base[:, W_OFF[1] : W_OFF[1] + 2048] = _chunk_pt(fc2_w.T, f16)
    wall_base[:, W_OFF[2] : W_OFF[2] + 1024] = _chunk_pt(fc3_w.T, f16)

    in_maps = []
    for c in range(NCORES):
        xk = np.ascontiguousarray(x[c * BL : (c + 1) * BL], dtype=f32)
        sx = xk.astype(f64).sum(axis=1)
        qx = (xk.astype(f64) ** 2).sum(axis=1)
        alpha1 = 1.0 + C2 * sx + C1
        k1 = C1 / alpha1
        de21 = C0 * qx + Cb * sx
        pk8 = pk8_base.copy()
        pk8[:, P8_ALPHA1] = alpha1
        pk8[:, P8_K1] = k1
        pk8[:, P8_DE21] = de21
        pk2 = pk2_base.copy()
        pk2[1, K2_LHS2 : K2_LHS2 + BL] = de21.astype(f16)
        wall = wall_base.copy()
        wall[:, XT_OFF : XT_OFF + XT_LEN] = _chunk_pt(xk.T.copy(), f16)
        in_maps.append({"pk8": pk8, "pk2": pk2, "wall": wall})
    return in_maps


def kernel(**inputs):
    from concourse.bass_utils import run_bass_kernel_spmd

    nc = get_nc()
    in_maps = host_prep(**inputs)
    res = run_bass_kernel_spmd(nc, in_maps, core_ids=list(range(NCORES)))
    out = np.concatenate([res.results[k]["outb"] for k in range(NCORES)], axis=0)
    return np.ascontiguousarray(out, dtype=np.float32)
